# revision 42
# baseline (speedup 1.0000x reference)
"""Multi-head causal attention (B=2, S=2048, D=1024, H=16) on 8 trn2 cores.

Sharding: core c -> batch b=c//4, head-group g=c%4 (heads 4g..4g+3).
Each core: Q/K/V projections for its heads from xT[b], causal attention in
transposed layout, row-parallel out-projection partial. Host sums the 4
partials per batch (bf16 device output, f32 accumulation) and adds bias.

Schedule notes (v5):
- PE pre-warm: dummy matmuls on a zeroed tile during the input-DMA lead-in
  so the HAM clock gate is at 8/8 when real matmuls start.
- Weights arrive as one packed DMA per tensor.
- Attention emits scores one k-group ahead of the PV matmuls (both heads
  interleaved) so the PE never waits on ACT's exp latency.
- Score/exp/PV ranges are trimmed to the causally-live columns; dead
  columns are never consumed, so no masking memsets are needed.
- One filler PSUM pool carries V-projection chunks, pair-1 q/k projection
  chunks and out-projection chunks; they drip into attention slots
  continuously so the PE never has a multi-us hole (V is LDW-bound and
  would serialize ~23us if run standalone).
- 1/l via reciprocal_approx_fast (single DVE op, SBUF-staged: custom DVE
  ops misread PSUM at a partition offset).
"""

import collections

import numpy as np

import concourse.bass as bass
import concourse.tile as tile
import concourse.mybir as mybir
from concourse import bacc
from concourse.bass_utils import run_bass_kernel_spmd

B, S, D, H, DH = 2, 2048, 1024, 16, 64
NCORES = 8
HPC = 4          # heads per core
PAIRS = 2        # head pairs per core
QT = 512         # q tile (free dim of scoresT / PV matmuls)
KB = 128         # k block (partition dim of scoresT)
NQT = S // QT    # 4
NKB = S // KB    # 16
DC = D // 128    # 8 contraction chunks for projections
NW = HPC * DH    # 256 projection output columns per core
SCALE = 1.0 / np.sqrt(DH)

F32 = mybir.dt.float32
BF = mybir.dt.bfloat16


def _build():
    nc = bacc.Bacc("TRN2", target_bir_lowering=False, debug=False, num_devices=NCORES)

    xT = nc.dram_tensor("xT", [D, S], BF, kind="ExternalInput").ap()
    # weights pre-packed on host: [128, DC*NW] with chunk i at cols i*NW
    wq = nc.dram_tensor("wq", [128, DC * NW], BF, kind="ExternalInput").ap()
    wk = nc.dram_tensor("wk", [128, DC * NW], BF, kind="ExternalInput").ap()
    wv = nc.dram_tensor("wv", [128, DC * NW], BF, kind="ExternalInput").ap()
    # wo packed: [128, 2*D] with pair p at cols p*D
    wo = nc.dram_tensor("wo", [128, PAIRS * D], BF, kind="ExternalInput").ap()
    tri = nc.dram_tensor("tri", [KB, KB], BF, kind="ExternalInput").ap()
    out = nc.dram_tensor("out", [S, D], BF, kind="ExternalOutput").ap()

    with tile.TileContext(nc) as tc, \
         tc.tile_pool(name="persist", bufs=1) as persist:
        # ---- persistent tiles ----
        qt_sb = [persist.tile([128, S], BF, name=f"qt{p}", tag=f"qt{p}") for p in range(PAIRS)]
        kt_sb = [persist.tile([128, S], BF, name=f"kt{p}", tag=f"kt{p}") for p in range(PAIRS)]
        # V' tiles: per s-block j, [128, 4*65]; head hl at cols 65*hl, ones col at 65*hl+64
        vt_sb = [persist.tile([128, HPC * (DH + 1)], BF, name=f"vt{j}", tag=f"vt{j}") for j in range(NKB)]
        ctx_sb = [persist.tile([128, S], BF, name=f"ctx{p}", tag=f"ctx{p}") for p in range(PAIRS)]
        tri_sb = persist.tile([KB, KB], BF, name="tri", tag="tri")


        xts = [persist.tile([128, S], BF, name=f"xts{i}", tag=f"xts{i}") for i in range(DC)]
        wq_sb = persist.tile([128, DC * NW], BF, name="wq", tag="wq")
        wk_sb = persist.tile([128, DC * NW], BF, name="wk", tag="wk")
        wv_sb = persist.tile([128, DC * NW], BF, name="wv", tag="wv")
        wo_sb = persist.tile([128, PAIRS * D], BF, name="wo", tag="wo")

        def wslice(w_all, i, lo, hi):
            return w_all[:, i * NW + lo:i * NW + hi]

        nc.sync.dma_start(tri_sb[:], tri[:])
        nc.sync.dma_start(xts[0][:], xT[0:128, :])
        nc.sync.dma_start(wq_sb[:], wq[:])
        nc.sync.dma_start(wk_sb[:], wk[:])
        for i in range(1, DC):
            nc.sync.dma_start(xts[i][:], xT[i * 128:(i + 1) * 128, :])
        nc.sync.dma_start(wv_sb[:], wv[:])
        nc.sync.dma_start(wo_sb[:], wo[:])

        # ---- PE warm-up while input DMAs land: tri lands first (~0.5us),
        # so it fuels the warm-up matmuls with no memset dependency ----
        with tc.tile_pool(name="warm", bufs=1, space="PSUM") as wps:
            wt = wps.tile([128, KB], F32, name="warm", tag="warm")
            for _ in range(40):
                nc.tensor.matmul(wt[:], tri_sb[:], tri_sb[:], start=True, stop=True)

        def proj_qk_chunked(p, pool):
            """q/k projection for pair p, D-chunk-outer so matmuls chase the
            xT DMAs chunk by chunk. st 0-2 chase the arriving chunk; st 3's
            matmuls for the previous chunk fill the DMA-wait slack (6+2
            psum banks; the filler pool still fits at the phase-B handoff)."""
            sts = (0, 1, 2)
            qps = {st: pool.tile([128, QT], F32, name=f"qps{st}", tag=f"qk{st}") for st in sts}
            kps = {st: pool.tile([128, QT], F32, name=f"kps{st}", tag=f"qk{3 + st}") for st in sts}
            q3 = pool.tile([128, QT], F32, name="qps3", tag="qk3q")
            k3 = pool.tile([128, QT], F32, name="kps3", tag="qk3k")

            def mm3(j):
                nc.tensor.matmul(
                    q3[:], wslice(wq_sb, j, p * 128, (p + 1) * 128),
                    xts[j][:, 3 * QT:4 * QT], start=(j == 0), stop=(j == DC - 1))
                nc.tensor.matmul(
                    k3[:], wslice(wk_sb, j, p * 128, (p + 1) * 128),
                    xts[j][:, 3 * QT:4 * QT], start=(j == 0), stop=(j == DC - 1))

            for i in range(DC):
                for st in sts:
                    nc.tensor.matmul(
                        qps[st][:], wslice(wq_sb, i, p * 128, (p + 1) * 128),
                        xts[i][:, st * QT:(st + 1) * QT],
                        start=(i == 0), stop=(i == DC - 1))
                for st in sts:
                    nc.tensor.matmul(
                        kps[st][:], wslice(wk_sb, i, p * 128, (p + 1) * 128),
                        xts[i][:, st * QT:(st + 1) * QT],
                        start=(i == 0), stop=(i == DC - 1))
                if i >= 1:
                    mm3(i - 1)
            mm3(DC - 1)
            for st in sts:
                nc.scalar.copy(qt_sb[p][:, st * QT:(st + 1) * QT], qps[st][:])
                nc.vector.tensor_copy(kt_sb[p][:, st * QT:(st + 1) * QT], kps[st][:])
            nc.scalar.copy(qt_sb[p][:, 3 * QT:4 * QT], q3[:])
            nc.vector.tensor_copy(kt_sb[p][:, 3 * QT:4 * QT], k3[:])

        def v_chunk(j, pool):
            """V projection + evac for one 128-seq block."""
            def emit():
                vp = pool.tile([128, HPC * DH], F32, name="vp", tag="fill")
                for i in range(DC):
                    nc.tensor.matmul(
                        vp[:], xts[i][:, j * 128:(j + 1) * 128],
                        wslice(wv_sb, i, 0, NW),
                        start=(i == 0), stop=(i == DC - 1))
                vt_view = vt_sb[j].rearrange("p (h e) -> p h e", h=HPC)
                nc.vector.tensor_copy(
                    vt_view[:, :, 0:DH], vp.rearrange("p (h e) -> p h e", h=HPC))
                nc.gpsimd.memset(vt_view[:, :, DH:DH + 1], 1.0)
            return emit

        def qk1_chunks(pool):
            """pair-1 q/k projection as 8 filler chunks, ordered so pair-1's
            ascending q-tiles find their tiles ready just in time."""
            def mk(which, st):
                def emit():
                    pp = pool.tile([128, QT], F32, name="qk1", tag="fill")
                    w = wq_sb if which == 0 else wk_sb
                    dst = qt_sb[1] if which == 0 else kt_sb[1]
                    for i in range(DC):
                        nc.tensor.matmul(
                            pp[:], wslice(w, i, 128, 256),
                            xts[i][:, st * QT:(st + 1) * QT],
                            start=(i == 0), stop=(i == DC - 1))
                    nc.vector.tensor_copy(dst[:, st * QT:(st + 1) * QT], pp[:])
                return emit
            order = [(0, 0), (1, 0), (1, 1), (0, 1), (1, 2), (0, 2), (1, 3), (0, 3)]
            return [mk(w, st) for (w, st) in order]

        def out_chunks(qt_i, pool, ph3sb):
            """partial out-projection for one q tile as 8 filler chunks
            (qb x nh); evac alternates ACT/DVE. Bias applied on the host."""
            chunks = []
            for qb in range(qt_i * 4, qt_i * 4 + 4):
                osref = {}
                def mk(qb, nh, osref):
                    def emit():
                        if nh == 0:
                            osref['t'] = ph3sb.tile([128, D], BF, name="os", tag="os")
                        os_ = osref['t']
                        op = pool.tile([128, 512], F32, name="op", tag="fill")
                        for p in range(PAIRS):
                            nc.tensor.matmul(
                                op[:], ctx_sb[p][:, qb * 128:(qb + 1) * 128],
                                wo_sb[:, p * D + nh * 512:p * D + (nh + 1) * 512],
                                start=(p == 0), stop=(p == PAIRS - 1))
                        dst = os_[:, nh * 512:(nh + 1) * 512]
                        nc.vector.tensor_copy(dst, op[:])
                        if nh == 1:
                            nc.sync.dma_start(out[qb * 128:(qb + 1) * 128, :], os_[:])
                    return emit
                for nh in range(2):
                    chunks.append(mk(qb, nh, osref))
            return chunks

        class Dripper:
            """Emit filler chunks at `rate` chunks per attention slot; when
            the queue runs dry, emit a HAM-warmth junk matmul instead."""
            def __init__(self, fallback):
                self.q = collections.deque()
                self.acc = 0.0
                self.fallback = fallback
            def add(self, chunks):
                self.q.extend(chunks)
            def slot(self, rate):
                self.acc += rate
                while self.acc >= 1.0:
                    if self.q:
                        self.q.popleft()()
                    else:
                        self.fallback()
                        self.acc = 0.0
                        return
                    self.acc -= 1.0
            def drain(self):
                while self.q:
                    self.q.popleft()()

        def attention_qt(p, qt_i, scps, ctxps, att, attsm, drip, rate):
            """Causal attention for both heads of pair p on q tile qt_i."""
            q0 = qt_i * QT
            nkb = 4 * (qt_i + 1)
            gs = list(range(0, nkb, 2))
            cps = [ctxps.tile([DH + 1, QT], F32, name=f"cps{h}", tag=f"cps{h}")
                   for h in range(2)]
            pts = {}

            def live0(kb):
                d = kb - 4 * qt_i
                return KB * d if d > 0 else 0

            def emit_scores(g0):
                for h in range(2):
                    r0, r1 = h * 64, h * 64 + 64
                    sp = scps.tile([128, 2 * QT], F32, name="sp", tag="sp")
                    for u in range(2):
                        kb = g0 + u
                        c0 = live0(kb)
                        nc.tensor.matmul(
                            sp[:, u * QT + c0:(u + 1) * QT],
                            kt_sb[p][r0:r1, kb * KB:(kb + 1) * KB],
                            qt_sb[p][r0:r1, q0 + c0:q0 + QT],
                            start=True, stop=True)
                    c0g = live0(g0)
                    pt = att.tile([128, 2 * QT], BF, name="pt", tag="pt")
                    nc.scalar.activation(
                        pt[:, c0g:2 * QT], sp[:, c0g:2 * QT],
                        mybir.ActivationFunctionType.Exp, scale=float(SCALE))
                    if g0 == 4 * qt_i:          # blocks d=0, d=1
                        for off in (0, QT + KB):
                            nc.vector.tensor_mul(
                                pt[:, off:off + KB], pt[:, off:off + KB], tri_sb[:])
                    elif g0 == 4 * qt_i + 2:    # blocks d=2, d=3
                        for off in (2 * KB, QT + 3 * KB):
                            nc.vector.tensor_mul(
                                pt[:, off:off + KB], pt[:, off:off + KB], tri_sb[:])
                    pts[(h, g0)] = pt

            def emit_pv(g0):
                for h in range(2):
                    hl = 2 * p + h
                    pt = pts.pop((h, g0))
                    for u in range(2):
                        kb = g0 + u
                        c0 = live0(kb)
                        nc.tensor.matmul(
                            cps[h][:, c0:QT],
                            vt_sb[kb][:, hl * (DH + 1):(hl + 1) * (DH + 1)],
                            pt[:, u * QT + c0:(u + 1) * QT],
                            start=(kb == 0), stop=(kb == nkb - 1))

            emit_scores(gs[0])
            for i, g0 in enumerate(gs):
                drip.slot(rate)
                if i + 1 < len(gs):
                    emit_scores(gs[i + 1])
                emit_pv(g0)

            # normalize: ctx = cps[0:64] * (1/l) with l = cps row 64
            for h in range(2):
                r0 = h * 64
                l_sb = attsm.tile([1, QT], F32, name="l_sb", tag="l")
                nc.vector.tensor_copy(l_sb[:], cps[h][DH:DH + 1, :])
                r_sb = attsm.tile([1, QT], F32, name="r_sb", tag="r")
                nc.vector.reciprocal_approx_fast(out=r_sb[:], in_=l_sb[:])
                rb = attsm.tile([64, QT], F32, name="rb", tag="rb")
                nc.gpsimd.partition_broadcast(rb[:], r_sb[:])
                nc.vector.tensor_mul(
                    ctx_sb[p][r0:r0 + 64, q0:q0 + QT], cps[h][0:DH, :], rb[:])

        # phase A: q/k pair 0, chunk-pipelined against the input DMAs
        with tc.tile_pool(name="qk0ps", bufs=1, space="PSUM") as qk0ps:
            proj_qk_chunked(0, qk0ps)

        # phase B: everything else under one filler pool + attention pools
        with tc.tile_pool(name="att", bufs=6) as att, \
             tc.tile_pool(name="attsm", bufs=3) as attsm, \
             tc.tile_pool(name="ph3sb", bufs=3) as ph3sb, \
             tc.tile_pool(name="scps", bufs=2, space="PSUM") as scps, \
             tc.tile_pool(name="ctxps", bufs=1, space="PSUM") as ctxps, \
             tc.tile_pool(name="fillps", bufs=2, space="PSUM") as fillps:

            def junk_chunk():
                # ~0.5us of dependency-free PE work to keep the HAM clock warm
                wt2 = fillps.tile([128, KB], F32, name="jmm", tag="fill")
                for _ in range(10):
                    nc.tensor.matmul(wt2[:], tri_sb[:], tri_sb[:], start=True, stop=True)

            drip = Dripper(junk_chunk)
            # V chunks drip just-in-time (qt0's rate covers blocks 0..3
            # inside its first slot, before the first PV needs them)
            drip.add([v_chunk(j, fillps) for j in range(NKB)])
            drip.add(qk1_chunks(fillps))

            # pair-0 attention; V then qk1 chunks drip in as fillers
            for qt_i, rate in zip(range(NQT), (2.0, 1.5, 0.8, 0.8)):
                attention_qt(0, qt_i, scps, ctxps, att, attsm, drip, rate)

            # pair-1 attention; leftovers then out-projection chunks
            for qt_i, rate in zip(range(NQT), (1.2, 2.0, 1.35, 1.2)):
                attention_qt(1, qt_i, scps, ctxps, att, attsm, drip, rate)
                drip.add(out_chunks(qt_i, fillps, ph3sb))
            # bridge the last normalize->outproj latency so the PE stays warm
            for _ in range(5):
                junk_chunk()
            drip.drain()

    nc.compile()
    return nc


_NC = None
PROFILE = False
TRACE_CORES = (0,)
LAST_RESULT = None


def _get_nc():
    global _NC
    if _NC is None:
        _NC = _build()
    return _NC


def kernel(x, Wq, Wk, Wv, Wo, bo):
    x = np.asarray(x, dtype=np.float32)
    Wq = np.asarray(Wq, dtype=np.float32)
    Wk = np.asarray(Wk, dtype=np.float32)
    Wv = np.asarray(Wv, dtype=np.float32)
    Wo = np.asarray(Wo, dtype=np.float32)
    bo = np.asarray(bo, dtype=np.float32)

    nc = _get_nc()

    in_maps = _prepare_in_maps(x, Wq, Wk, Wv, Wo)

    global LAST_RESULT
    kw = {}
    if PROFILE:
        kw = dict(trace=True, trace_cores=list(TRACE_CORES))
    res = run_bass_kernel_spmd(nc, in_maps, core_ids=list(range(NCORES)), **kw)
    LAST_RESULT = res

    out = np.zeros((B, S, D), np.float32)
    for c in range(NCORES):
        b = c // 4
        out[b] += res.results[c]["out"].astype(np.float32)
    out += bo.astype(np.float32)
    return out


def _pack_w(w):
    """[D, N] -> [128, DC*N] with D-chunk i at cols [i*N, (i+1)*N)."""
    Dd, N = w.shape
    return np.ascontiguousarray(
        w.reshape(Dd // 128, 128, N).transpose(1, 0, 2).reshape(128, -1))


def _prepare_in_maps(x, Wq, Wk, Wv, Wo):
    kk = np.arange(KB)[:, None]
    qq = np.arange(KB)[None, :]
    import ml_dtypes
    bf16 = ml_dtypes.bfloat16
    tri = (kk <= qq).astype(bf16)

    xTs = [np.ascontiguousarray(x[b].T).astype(bf16) for b in range(B)]

    in_maps = []
    for c in range(NCORES):
        b, g = divmod(c, 4)
        cs = slice(g * HPC * DH, (g + 1) * HPC * DH)
        in_maps.append({
            "xT": xTs[b],
            "wq": _pack_w(Wq[:, cs]).astype(bf16),
            "wk": _pack_w(Wk[:, cs]).astype(bf16),
            "wv": _pack_w(Wv[:, cs]).astype(bf16),
            "wo": _pack_w(Wo[cs, :]).astype(bf16),
            "tri": tri,
        })
    return in_maps


# revision 46
# speedup vs baseline: 1.0076x; 1.0076x over previous
"""Multi-head causal attention (B=2, S=2048, D=1024, H=16) on 8 trn2 cores.

Sharding: core c -> batch b=c//4, head-group g=c%4 (heads 4g..4g+3).
Each core: Q/K/V projections for its heads from xT[b], causal attention in
transposed layout, row-parallel out-projection partial. Host sums the 4
partials per batch (bf16 device output, f32 accumulation) and adds bias.

Schedule notes (v5):
- PE pre-warm: dummy matmuls on a zeroed tile during the input-DMA lead-in
  so the HAM clock gate is at 8/8 when real matmuls start.
- Weights arrive as one packed DMA per tensor.
- Attention emits scores one k-group ahead of the PV matmuls (both heads
  interleaved) so the PE never waits on ACT's exp latency.
- Score/exp/PV ranges are trimmed to the causally-live columns; dead
  columns are never consumed, so no masking memsets are needed.
- One filler PSUM pool carries V-projection chunks, pair-1 q/k projection
  chunks and out-projection chunks; they drip into attention slots
  continuously so the PE never has a multi-us hole (V is LDW-bound and
  would serialize ~23us if run standalone).
- 1/l via reciprocal_approx_fast (single DVE op, SBUF-staged: custom DVE
  ops misread PSUM at a partition offset).
"""

import collections

import numpy as np

import concourse.bass as bass
import concourse.tile as tile
import concourse.mybir as mybir
from concourse import bacc
from concourse.bass_utils import run_bass_kernel_spmd

B, S, D, H, DH = 2, 2048, 1024, 16, 64
NCORES = 8
HPC = 4          # heads per core
PAIRS = 2        # head pairs per core
QT = 512         # q tile (free dim of scoresT / PV matmuls)
KB = 128         # k block (partition dim of scoresT)
NQT = S // QT    # 4
NKB = S // KB    # 16
DC = D // 128    # 8 contraction chunks for projections
NW = HPC * DH    # 256 projection output columns per core
SCALE = 1.0 / np.sqrt(DH)

F32 = mybir.dt.float32
BF = mybir.dt.bfloat16


def _build():
    nc = bacc.Bacc("TRN2", target_bir_lowering=False, debug=False, num_devices=NCORES)

    xT = nc.dram_tensor("xT", [D, S], BF, kind="ExternalInput").ap()
    # weights pre-packed on host: [128, DC*NW] with chunk i at cols i*NW
    wq = nc.dram_tensor("wq", [128, DC * NW], BF, kind="ExternalInput").ap()
    wk = nc.dram_tensor("wk", [128, DC * NW], BF, kind="ExternalInput").ap()
    wv = nc.dram_tensor("wv", [128, DC * NW], BF, kind="ExternalInput").ap()
    # wo packed: [128, 2*D] with pair p at cols p*D
    wo = nc.dram_tensor("wo", [128, PAIRS * D], BF, kind="ExternalInput").ap()
    tri = nc.dram_tensor("tri", [KB, KB], BF, kind="ExternalInput").ap()
    out = nc.dram_tensor("out", [S, D], BF, kind="ExternalOutput").ap()

    with tile.TileContext(nc) as tc, \
         tc.tile_pool(name="persist", bufs=1) as persist:
        # ---- persistent tiles ----
        qt_sb = [persist.tile([128, S], BF, name=f"qt{p}", tag=f"qt{p}") for p in range(PAIRS)]
        kt_sb = [persist.tile([128, S], BF, name=f"kt{p}", tag=f"kt{p}") for p in range(PAIRS)]
        # V' tiles: per s-block j, [128, 4*65]; head hl at cols 65*hl, ones col at 65*hl+64
        vt_sb = [persist.tile([128, HPC * (DH + 1)], BF, name=f"vt{j}", tag=f"vt{j}") for j in range(NKB)]
        ctx_sb = [persist.tile([128, S], BF, name=f"ctx{p}", tag=f"ctx{p}") for p in range(PAIRS)]
        tri_sb = persist.tile([KB, KB], BF, name="tri", tag="tri")


        xts = [persist.tile([128, S], BF, name=f"xts{i}", tag=f"xts{i}") for i in range(DC)]
        wq_sb = persist.tile([128, DC * NW], BF, name="wq", tag="wq")
        wk_sb = persist.tile([128, DC * NW], BF, name="wk", tag="wk")
        wv_sb = persist.tile([128, DC * NW], BF, name="wv", tag="wv")
        wo_sb = persist.tile([128, PAIRS * D], BF, name="wo", tag="wo")

        def wslice(w_all, i, lo, hi):
            return w_all[:, i * NW + lo:i * NW + hi]

        nc.sync.dma_start(tri_sb[:], tri[:])
        nc.sync.dma_start(xts[0][:], xT[0:128, :])
        nc.sync.dma_start(wq_sb[:], wq[:])
        nc.sync.dma_start(wk_sb[:], wk[:])
        for i in range(1, DC):
            nc.sync.dma_start(xts[i][:], xT[i * 128:(i + 1) * 128, :])
        nc.sync.dma_start(wv_sb[:], wv[:])
        nc.sync.dma_start(wo_sb[:], wo[:])

        # ---- PE warm-up while input DMAs land: tri lands first (~0.5us),
        # so it fuels the warm-up matmuls with no memset dependency. The
        # warm tile stays alive through phase A (6+1 banks) so the chunk
        # chase can sprinkle keep-warm matmuls into its DMA-wait slack ----
        warm_ctx = tc.tile_pool(name="warm", bufs=1, space="PSUM")
        wps = warm_ctx.__enter__()
        wt = wps.tile([128, KB], F32, name="warm", tag="warm")
        for _ in range(40):
            nc.tensor.matmul(wt[:], tri_sb[:], tri_sb[:], start=True, stop=True)

        def proj_qk_chunked(p, pool):
            """q/k projection for pair p, D-chunk-outer so matmuls chase the
            xT DMAs chunk by chunk. A 3-tile pass then a 1-tile pass (6+2
            psum banks) so the filler pool can coexist (no pool-handoff
            stall into phase B) while the DMA-chase stays nearly PE-dense."""
            for sts in ((0, 1, 2), (3,)):
                qps = {st: pool.tile([128, QT], F32, name=f"qps{st}", tag=f"qk{st % 3}") for st in sts}
                kps = {st: pool.tile([128, QT], F32, name=f"kps{st}", tag=f"qk{3 + st % 3}") for st in sts}
                for i in range(DC):
                    for st in sts:
                        nc.tensor.matmul(
                            qps[st][:], wslice(wq_sb, i, p * 128, (p + 1) * 128),
                            xts[i][:, st * QT:(st + 1) * QT],
                            start=(i == 0), stop=(i == DC - 1))
                    for st in sts:
                        nc.tensor.matmul(
                            kps[st][:], wslice(wk_sb, i, p * 128, (p + 1) * 128),
                            xts[i][:, st * QT:(st + 1) * QT],
                            start=(i == 0), stop=(i == DC - 1))
                    if sts[0] == 0:
                        # hold the HAM clock warm across each chunk's DMA wait
                        for _ in range(2):
                            nc.tensor.matmul(wt[:], tri_sb[:], tri_sb[:],
                                             start=True, stop=True)
                for st in sts:
                    nc.scalar.copy(qt_sb[p][:, st * QT:(st + 1) * QT], qps[st][:])
                    nc.vector.tensor_copy(kt_sb[p][:, st * QT:(st + 1) * QT], kps[st][:])

        def v_chunk(j, pool):
            """V projection + evac for one 128-seq block."""
            def emit():
                vp = pool.tile([128, HPC * DH], F32, name="vp", tag="fill")
                for i in range(DC):
                    nc.tensor.matmul(
                        vp[:], xts[i][:, j * 128:(j + 1) * 128],
                        wslice(wv_sb, i, 0, NW),
                        start=(i == 0), stop=(i == DC - 1))
                vt_view = vt_sb[j].rearrange("p (h e) -> p h e", h=HPC)
                nc.vector.tensor_copy(
                    vt_view[:, :, 0:DH], vp.rearrange("p (h e) -> p h e", h=HPC))
                nc.gpsimd.memset(vt_view[:, :, DH:DH + 1], 1.0)
            return emit

        def qk1_chunks(pool):
            """pair-1 q/k projection as 8 filler chunks, ordered so pair-1's
            ascending q-tiles find their tiles ready just in time."""
            def mk(which, st):
                def emit():
                    pp = pool.tile([128, QT], F32, name="qk1", tag="fill")
                    w = wq_sb if which == 0 else wk_sb
                    dst = qt_sb[1] if which == 0 else kt_sb[1]
                    for i in range(DC):
                        nc.tensor.matmul(
                            pp[:], wslice(w, i, 128, 256),
                            xts[i][:, st * QT:(st + 1) * QT],
                            start=(i == 0), stop=(i == DC - 1))
                    nc.vector.tensor_copy(dst[:, st * QT:(st + 1) * QT], pp[:])
                return emit
            order = [(0, 0), (1, 0), (1, 1), (0, 1), (1, 2), (0, 2), (1, 3), (0, 3)]
            return [mk(w, st) for (w, st) in order]

        def out_chunks(qt_i, pool, ph3sb):
            """partial out-projection for one q tile as 8 filler chunks
            (qb x nh); evac alternates ACT/DVE. Bias applied on the host."""
            chunks = []
            for qb in range(qt_i * 4, qt_i * 4 + 4):
                osref = {}
                def mk(qb, nh, osref):
                    def emit():
                        if nh == 0:
                            osref['t'] = ph3sb.tile([128, D], BF, name="os", tag="os")
                        os_ = osref['t']
                        op = pool.tile([128, 512], F32, name="op", tag="fill")
                        for p in range(PAIRS):
                            nc.tensor.matmul(
                                op[:], ctx_sb[p][:, qb * 128:(qb + 1) * 128],
                                wo_sb[:, p * D + nh * 512:p * D + (nh + 1) * 512],
                                start=(p == 0), stop=(p == PAIRS - 1))
                        dst = os_[:, nh * 512:(nh + 1) * 512]
                        nc.vector.tensor_copy(dst, op[:])
                        if nh == 1:
                            nc.sync.dma_start(out[qb * 128:(qb + 1) * 128, :], os_[:])
                    return emit
                for nh in range(2):
                    chunks.append(mk(qb, nh, osref))
            return chunks

        class Dripper:
            """Emit filler chunks at `rate` chunks per attention slot; when
            the queue runs dry, emit a HAM-warmth junk matmul instead."""
            def __init__(self, fallback):
                self.q = collections.deque()
                self.acc = 0.0
                self.fallback = fallback
            def add(self, chunks):
                self.q.extend(chunks)
            def slot(self, rate):
                self.acc += rate
                while self.acc >= 1.0:
                    if self.q:
                        self.q.popleft()()
                    else:
                        self.fallback()
                        self.acc = 0.0
                        return
                    self.acc -= 1.0
            def drain(self):
                while self.q:
                    self.q.popleft()()

        def attention_qt(p, qt_i, scps, ctxps, att, attsm, drip, rate):
            """Causal attention for both heads of pair p on q tile qt_i."""
            q0 = qt_i * QT
            nkb = 4 * (qt_i + 1)
            gs = list(range(0, nkb, 2))
            cps = [ctxps.tile([DH + 1, QT], F32, name=f"cps{h}", tag=f"cps{h}")
                   for h in range(2)]
            pts = {}

            def live0(kb):
                d = kb - 4 * qt_i
                return KB * d if d > 0 else 0

            def emit_scores(g0):
                for h in range(2):
                    r0, r1 = h * 64, h * 64 + 64
                    sp = scps.tile([128, 2 * QT], F32, name="sp", tag="sp")
                    for u in range(2):
                        kb = g0 + u
                        c0 = live0(kb)
                        nc.tensor.matmul(
                            sp[:, u * QT + c0:(u + 1) * QT],
                            kt_sb[p][r0:r1, kb * KB:(kb + 1) * KB],
                            qt_sb[p][r0:r1, q0 + c0:q0 + QT],
                            start=True, stop=True)
                    c0g = live0(g0)
                    pt = att.tile([128, 2 * QT], BF, name="pt", tag="pt")
                    nc.scalar.activation(
                        pt[:, c0g:2 * QT], sp[:, c0g:2 * QT],
                        mybir.ActivationFunctionType.Exp, scale=float(SCALE))
                    if g0 == 4 * qt_i:          # blocks d=0, d=1
                        for off in (0, QT + KB):
                            nc.vector.tensor_mul(
                                pt[:, off:off + KB], pt[:, off:off + KB], tri_sb[:])
                    elif g0 == 4 * qt_i + 2:    # blocks d=2, d=3
                        for off in (2 * KB, QT + 3 * KB):
                            nc.vector.tensor_mul(
                                pt[:, off:off + KB], pt[:, off:off + KB], tri_sb[:])
                    pts[(h, g0)] = pt

            def emit_pv(g0):
                for h in range(2):
                    hl = 2 * p + h
                    pt = pts.pop((h, g0))
                    for u in range(2):
                        kb = g0 + u
                        c0 = live0(kb)
                        nc.tensor.matmul(
                            cps[h][:, c0:QT],
                            vt_sb[kb][:, hl * (DH + 1):(hl + 1) * (DH + 1)],
                            pt[:, u * QT + c0:(u + 1) * QT],
                            start=(kb == 0), stop=(kb == nkb - 1))

            emit_scores(gs[0])
            for i, g0 in enumerate(gs):
                drip.slot(rate)
                if i + 1 < len(gs):
                    emit_scores(gs[i + 1])
                emit_pv(g0)

            # normalize: ctx = cps[0:64] * (1/l) with l = cps row 64
            for h in range(2):
                r0 = h * 64
                l_sb = attsm.tile([1, QT], F32, name="l_sb", tag="l")
                nc.vector.tensor_copy(l_sb[:], cps[h][DH:DH + 1, :])
                r_sb = attsm.tile([1, QT], F32, name="r_sb", tag="r")
                nc.vector.reciprocal_approx_fast(out=r_sb[:], in_=l_sb[:])
                rb = attsm.tile([64, QT], F32, name="rb", tag="rb")
                nc.gpsimd.partition_broadcast(rb[:], r_sb[:])
                nc.vector.tensor_mul(
                    ctx_sb[p][r0:r0 + 64, q0:q0 + QT], cps[h][0:DH, :], rb[:])

        # phase A: q/k pair 0, chunk-pipelined against the input DMAs
        with tc.tile_pool(name="qk0ps", bufs=1, space="PSUM") as qk0ps:
            proj_qk_chunked(0, qk0ps)
        warm_ctx.__exit__(None, None, None)

        # phase B: everything else under one filler pool + attention pools
        with tc.tile_pool(name="att", bufs=6) as att, \
             tc.tile_pool(name="attsm", bufs=3) as attsm, \
             tc.tile_pool(name="ph3sb", bufs=3) as ph3sb, \
             tc.tile_pool(name="scps", bufs=2, space="PSUM") as scps, \
             tc.tile_pool(name="ctxps", bufs=1, space="PSUM") as ctxps, \
             tc.tile_pool(name="fillps", bufs=2, space="PSUM") as fillps:

            def junk_chunk():
                # ~0.5us of dependency-free PE work to keep the HAM clock warm
                wt2 = fillps.tile([128, KB], F32, name="jmm", tag="fill")
                for _ in range(10):
                    nc.tensor.matmul(wt2[:], tri_sb[:], tri_sb[:], start=True, stop=True)

            drip = Dripper(junk_chunk)
            # V chunks drip just-in-time (qt0's rate covers blocks 0..3
            # inside its first slot, before the first PV needs them)
            drip.add([v_chunk(j, fillps) for j in range(NKB)])
            drip.add(qk1_chunks(fillps))

            # pair-0 attention; V then qk1 chunks drip in as fillers
            for qt_i, rate in zip(range(NQT), (2.0, 1.5, 0.8, 0.8)):
                attention_qt(0, qt_i, scps, ctxps, att, attsm, drip, rate)

            # pair-1 attention; leftovers then out-projection chunks
            for qt_i, rate in zip(range(NQT), (1.2, 2.0, 1.35, 1.2)):
                attention_qt(1, qt_i, scps, ctxps, att, attsm, drip, rate)
                drip.add(out_chunks(qt_i, fillps, ph3sb))
            # bridge the last normalize->outproj latency so the PE stays warm
            for _ in range(5):
                junk_chunk()
            drip.drain()

    nc.compile()
    return nc


_NC = None
PROFILE = False
TRACE_CORES = (0,)
LAST_RESULT = None


def _get_nc():
    global _NC
    if _NC is None:
        _NC = _build()
    return _NC


def kernel(x, Wq, Wk, Wv, Wo, bo):
    x = np.asarray(x, dtype=np.float32)
    Wq = np.asarray(Wq, dtype=np.float32)
    Wk = np.asarray(Wk, dtype=np.float32)
    Wv = np.asarray(Wv, dtype=np.float32)
    Wo = np.asarray(Wo, dtype=np.float32)
    bo = np.asarray(bo, dtype=np.float32)

    nc = _get_nc()

    in_maps = _prepare_in_maps(x, Wq, Wk, Wv, Wo)

    global LAST_RESULT
    kw = {}
    if PROFILE:
        kw = dict(trace=True, trace_cores=list(TRACE_CORES))
    res = run_bass_kernel_spmd(nc, in_maps, core_ids=list(range(NCORES)), **kw)
    LAST_RESULT = res

    out = np.zeros((B, S, D), np.float32)
    for c in range(NCORES):
        b = c // 4
        out[b] += res.results[c]["out"].astype(np.float32)
    out += bo.astype(np.float32)
    return out


def _pack_w(w):
    """[D, N] -> [128, DC*N] with D-chunk i at cols [i*N, (i+1)*N)."""
    Dd, N = w.shape
    return np.ascontiguousarray(
        w.reshape(Dd // 128, 128, N).transpose(1, 0, 2).reshape(128, -1))


def _prepare_in_maps(x, Wq, Wk, Wv, Wo):
    kk = np.arange(KB)[:, None]
    qq = np.arange(KB)[None, :]
    import ml_dtypes
    bf16 = ml_dtypes.bfloat16
    tri = (kk <= qq).astype(bf16)

    xTs = [np.ascontiguousarray(x[b].T).astype(bf16) for b in range(B)]

    in_maps = []
    for c in range(NCORES):
        b, g = divmod(c, 4)
        cs = slice(g * HPC * DH, (g + 1) * HPC * DH)
        in_maps.append({
            "xT": xTs[b],
            "wq": _pack_w(Wq[:, cs]).astype(bf16),
            "wk": _pack_w(Wk[:, cs]).astype(bf16),
            "wv": _pack_w(Wv[:, cs]).astype(bf16),
            "wo": _pack_w(Wo[cs, :]).astype(bf16),
            "tri": tri,
        })
    return in_maps
